# revision 68
# baseline (speedup 1.0000x reference)
"""MoCo loss (InfoNCE over a 65536-entry queue + proto-NCE over 50000
k-means centroids) on 8 Trainium2 NeuronCores.

Strategy: the heavy work is two matmuls, Z_q @ queue.T (256x512x65536)
and Z_q @ centroids.T (256x512x50000).  We shard the tables by row
across the 8 cores and replicate Z_q.

Per core:
  part 2 (centroid shard, batch -> partitions, centroids -> free):
    - fp8 e4m3 (scale 64) inputs, DoubleRow matmuls (2x PE throughput)
    - s2*4096 exported to DRAM as fp16 (argmax + exclusion gather on
      host; centroids are L2-normalized so argmin ||c||^2 - 2 s ==
      argmax s).  e4m3 shifts the proto-term argmax for ~30/256 rows
      (near-ties); measured end-to-end loss impact ~1e-3 relative.
  part 1 (queue shard, fp16, queue rows -> partitions, batch -> free):
    - s1 = q_shard @ Z_q.T                      (PE)
    - row-max + (s1[:,0] >= rowmax) on fp32 PSUM -> exact accuracy
    - exp(s1/T) to fp16 (no shift needed: s/T in [-4, 4])      (ACT)
    - ones-matmul partition sum -> per-batch partial sum-of-exp (PE)

All DMA goes through the sync-engine HWDGE ring (the gpsimd SWDGE path
costs ~2.7us of Q7 descriptor generation per transfer and serializes).
A burst of dummy ones-matmuls at the head warms the PE HAM clock gate
(cold PE runs at 1.2 GHz for the first ~3.4us of activity) while the
first DMAs are still in flight.

The host combines the tiny per-core partials (logsumexp merge, accuracy
count, global argmax, 513-wide proto softmax).
"""

import os
import numpy as np
import ml_dtypes

B, C = 256, 512
QUEUE, NCL, NNEG = 65536, 50000, 512
INFO_TEMP = 0.07
PROTO_FACTOR = 0.5
NCORES = 8
QSH = QUEUE // NCORES          # 8192 queue rows per core
CSH = NCL // NCORES            # 6250 centroid rows per core
CCH = 13                       # cT matmul chunks
CW = 512                       # cols per chunk (last chunk computes 128)
CWL = 128                      # last-chunk matmul width (6250 <= 12*512+128)
S2W = 12 * CW + CWL            # 6272 exported s2 columns
CPAIR = 7                      # cT DMA transfers (2 chunks each, 14th is pad)
KSUB = C // 128                # 4 fp16 contraction subtiles
NT = 32                        # part-1 tiles (256 queue rows each)
QCHUNK = 8                     # qT DMA chunks (1 MiB each)
JW = QSH // QCHUNK             # 1024
F8SCALE = 64.0                 # e4m3 quantization scale
S2SCALE = F8SCALE * F8SCALE    # s2 output carries this factor
NWARM = 48                     # PE warmup matmuls: must keep the PE busy
                               # from ~6.1us (gpsimd memset of ones) until
                               # part-2 data lands (~11.4us) — an idle gap
                               # before HAM fires leaves part 2 at 1.2GHz
# tiles whose subtile-1 sum also goes to gpsimd (spread out: two adds on
# one tile costs gpsimd 1.34us vs the PE's 0.87us/tile cadence)
GP_Q1 = frozenset(range(0, 30, 3))

_CACHE = {}

# exec time of the last device run (ns), populated when tracing is on
last_exec_time_ns = None


def _build():
    import concourse.bass as bass
    import concourse.tile as tile
    from concourse import bacc, mybir

    dt = mybir.dt
    DR = mybir.MatmulPerfMode.DoubleRow
    nc = bacc.Bacc(
        "TRN2", target_bir_lowering=False, debug=False, num_devices=NCORES
    )

    # ---- DRAM I/O (partition-major so every DMA is a flat [128, N]) ----
    # Each extra DMA stalls the HWDGE ring ~0.9us for its completion-sem
    # write, so zq8 rides with cT pair 0 and zqT with qT chunk 0.
    c8_d = nc.dram_tensor(
        "c8", [128, 4 * B + CPAIR * 8 * CW], dt.float8e4, kind="ExternalInput"
    ).ap()  # zq8 [s,i,b] ++ per-pair [j(chunk),s,i,col] blocks
    q16_d = nc.dram_tensor(
        "q16", [128, KSUB * B + QCHUNK * KSUB * JW], dt.float16,
        kind="ExternalInput"
    ).ap()  # zqT [s,b] ++ qT chunks [s,jw] blocks

    # [128, B+1]: cols 0..B-1 = per-batch partial sum-of-exp (all rows
    # identical); col B = per-partition accuracy count.  One flat DMA —
    # a separate [128]-column output pays a ~6us completion-latency tail.
    p1_d = nc.dram_tensor("p1", [128, B + 1], dt.float32, kind="ExternalOutput").ap()
    s2_d = nc.dram_tensor(
        "s2", [2, 128, S2W], dt.float16, kind="ExternalOutput"
    ).ap()

    with tile.TileContext(nc) as tc:
        with (
            tc.tile_pool(name="const", bufs=1) as cpool,
            tc.tile_pool(name="work", bufs=4) as wpool,
            tc.tile_pool(name="ps1", bufs=3, space="PSUM") as ps1,
            tc.tile_pool(name="psum1s", bufs=1, space="PSUM") as ps1s,
            tc.tile_pool(name="ps2", bufs=4, space="PSUM") as ps2,
        ):
            # ---- constants + PE warmup (runs while DMAs stream) ----
            # gpsimd exits the entry barrier ~1.5us before DVE, so the ones
            # memset (and with it the PE warmup) starts that much earlier —
            # HAM then un-throttles before the first real matmul
            ones_sb = cpool.tile([128, 128], dt.float16)
            nc.gpsimd.memset(ones_sb[:], 1.0)
            # warmup PSUM lives in the (part-1) ps1 pool so it doesn't eat
            # one of part-2's four copy-recycle slots
            warm_ps = ps1.tile([128, 512], dt.float32, tag="s1")
            for _ in range(NWARM):
                nc.tensor.matmul(warm_ps[:, :128], ones_sb[:], ones_sb[:],
                                 start=True, stop=True)

            # ---- resident SBUF tensors; all loads on the sync HWDGE ring.
            # Ring order == HBM priority: part-2 operands first, then the
            # part-1 operands, then (programmatically later) the outputs.
            NC0 = 4 * B + 8 * CW           # zq8 + cT pair 0
            NS0 = 4 * B + 4 * CW           # zq8 + chunk 0 only
            c0_sb = cpool.tile([128, NC0], dt.float8e4)
            # the first transfer runs at ramp rate (~150-250 GB/s), so land
            # zq8+chunk0 first and chunk1 as its own transfer
            nc.sync.dma_start(c0_sb[:, :NS0], c8_d[:, :NS0])
            nc.sync.dma_start(c0_sb[:, NS0:NC0], c8_d[:, NS0:NC0])
            zq8_sb = c0_sb[:, :4 * B].rearrange(
                "p (s i b) -> p s i b", s=2, i=2)
            cT8_sb = [c0_sb[:, 4 * B:].rearrange(
                "p (j s i w) -> p j s i w", j=2, s=2, i=2)]
            for p in range(1, CPAIR):
                t = cpool.tile([128, 8 * CW], dt.float8e4, tag=f"cT{p}")
                off = 4 * B + p * 8 * CW
                nc.sync.dma_start(t[:], c8_d[:, off:off + 8 * CW])
                cT8_sb.append(t.rearrange(
                    "p (j s i w) -> p j s i w", j=2, s=2, i=2))

            NQ0 = KSUB * B + KSUB * JW     # zqT + qT chunk 0
            q0_sb = cpool.tile([128, NQ0], dt.float16)
            nc.sync.dma_start(q0_sb[:], q16_d[:, :NQ0])
            zqT_sb = q0_sb[:, :KSUB * B].rearrange("p (s b) -> p s b", s=KSUB)
            qt_sb = [q0_sb[:, KSUB * B:].rearrange("p (s j) -> p s j", s=KSUB)]
            for h in range(1, QCHUNK):
                t = cpool.tile([128, KSUB, JW], dt.float16, tag=f"qt{h}")
                off = KSUB * B + h * KSUB * JW
                nc.sync.dma_start(t[:], q16_d[:, off:off + KSUB * JW])
                qt_sb.append(t)

            # ---- part 2: centroid shard, fp8 DoubleRow (argmax on host) ----
            s2_sb = cpool.tile([128, 2, S2W], dt.float16)

            # N=512 puts the DoubleRow moving operand at its 1024-element
            # max; each matmul output fills exactly one PSUM bank.  The
            # last chunk only has 106 live columns, so it runs at N=128.
            for ch in range(CCH):
                w = CW if ch < CCH - 1 else CWL
                col = ch * CW
                for bt in range(2):
                    s2_ps = ps2.tile([128, CW], dt.float32, tag="s2")
                    for s in range(2):
                        nc.tensor.matmul(
                            s2_ps[:, :w],
                            zq8_sb[:, s, :, bt * 128:(bt + 1) * 128],
                            cT8_sb[ch // 2][:, ch % 2, s, :, :w],
                            start=(s == 0),
                            stop=(s == 1),
                            perf_mode=DR,
                        )
                    # cast to fp16; strict DVE/ACT alternation (bt toggles
                    # every group — (ch+bt)%2 would put two consecutive
                    # groups on the same engine and stall the PSUM recycle)
                    if bt == 0:
                        nc.vector.tensor_copy(
                            s2_sb[:, bt, col:col + w], s2_ps[:, :w]
                        )
                    else:
                        nc.scalar.activation(
                            s2_sb[:, bt, col:col + w],
                            s2_ps[:, :w],
                            mybir.ActivationFunctionType.Copy,
                        )
            for bt in range(2):
                nc.sync.dma_start(s2_d[bt], s2_sb[:, bt])

            # ---- part 1: queue shard, 32 tiles of 256 rows ----
            rm_buf = cpool.tile([128, NT, 2], dt.float32)  # per-subtile row max
            s0_buf = cpool.tile([128, NT, 2], dt.float32)  # s1[:, batch 0]
            p1s_ps = ps1s.tile([128, B], dt.float32)       # sum-of-exp accum
            # subtile-0 exp sums accumulate on the (otherwise idle) gpsimd
            # engine; only subtile 1 goes through PE ones-matmuls.  fp16
            # partials stay < 64 per entry, so precision is ample.
            gacc = cpool.tile([128, B], dt.float16)
            nc.gpsimd.memset(gacc[:], 0.0)

            # ones-matmuls run two tiles behind the s1 matmuls so the
            # in-order PE never waits for ACT's exp of a recent tile
            LAG = 2
            exp_tiles = [None] * NT
            for t in range(NT):
                s1_ps = ps1.tile([128, 2, B], dt.float32, tag="s1")
                for q in range(2):
                    jt = t * 2 + q
                    h, off = divmod(jt * 128, JW)
                    for s in range(KSUB):
                        nc.tensor.matmul(
                            s1_ps[:, q, :],
                            qt_sb[h][:, s, off:off + 128],
                            zqT_sb[:, s, :],
                            start=(s == 0),
                            stop=(s == KSUB - 1),
                        )
                exp_t = wpool.tile([128, 2, B], dt.float16, tag="exp")
                exp_tiles[t] = exp_t
                nc.scalar.activation(
                    exp_t[:],
                    s1_ps[:],
                    mybir.ActivationFunctionType.Exp,
                    scale=1.0 / INFO_TEMP,
                )
                if t < NT - LAG:
                    # last LAG tiles' subtile sums go through the PE
                    # instead, so the final ones-matmul never waits on
                    # the trailing gpsimd chain.  gpsimd also absorbs the
                    # subtile-1 sums of the first NGQ1 tiles (engine
                    # balance: PE ~30.6us, gpsimd ~26.8us over part 1).
                    nc.gpsimd.tensor_add(gacc[:], gacc[:], exp_t[:, 0, :])
                    if t in GP_Q1:
                        nc.gpsimd.tensor_add(gacc[:], gacc[:], exp_t[:, 1, :])
                u = t - LAG
                if u >= 0 and u not in GP_Q1:
                    nc.tensor.matmul(
                        p1s_ps[:],
                        ones_sb[:],
                        exp_tiles[u][:, 1, :],
                        start=(u == 1),
                        stop=False,
                    )
                nc.vector.tensor_reduce(
                    rm_buf[:, t, :],
                    s1_ps[:],
                    axis=mybir.AxisListType.X,
                    op=mybir.AluOpType.max,
                )
                nc.vector.tensor_copy(s0_buf[:, t, :], s1_ps[:, :, 0])
            # tail: last LAG tiles' sums (they wait on the final exps), then
            # the gacc pass last — its wait on the trailing gpsimd add chain
            # overlaps the exp waits above
            for tl in range(NT - LAG, NT):
                for q in range(2):
                    nc.tensor.matmul(
                        p1s_ps[:],
                        ones_sb[:],
                        exp_tiles[tl][:, q, :],
                        start=False,
                        stop=False,
                    )
            nc.tensor.matmul(
                p1s_ps[:], ones_sb[:], gacc[:], start=False, stop=True
            )

            ge_buf = cpool.tile([128, NT * 2], dt.float32)
            nc.vector.tensor_tensor(
                ge_buf[:],
                s0_buf[:].rearrange("p t q -> p (t q)"),
                rm_buf[:].rearrange("p t q -> p (t q)"),
                mybir.AluOpType.is_ge,
            )
            p1_sb = cpool.tile([128, B + 1], dt.float32)
            nc.vector.tensor_reduce(
                p1_sb[:, B:], ge_buf[:], axis=mybir.AxisListType.X,
                op=mybir.AluOpType.add
            )
            nc.vector.tensor_copy(p1_sb[:, :B], p1s_ps[:])
            nc.sync.dma_start(p1_d[:], p1_sb[:])

    nc.compile()
    return nc


def _get_nc():
    if "nc" not in _CACHE:
        _CACHE["nc"] = _build()
    return _CACHE["nc"]


def _prep_inputs(Z_q, queue, centroids):
    """Host-side shard prep.

    fp16 tensors go partition-major with the contraction dim C split in
    KSUB=4 blocks of 128 (c = s*128 + k).  fp8 tensors use DoubleRow
    pairing [k, s, i, col] with c = s*256 + i*128 + k on BOTH operands.
    """
    e4 = ml_dtypes.float8_e4m3
    zqT = np.ascontiguousarray(
        Z_q.astype(np.float16).T.reshape(KSUB, 128, B).transpose(1, 0, 2)
    ).reshape(128, KSUB * B)  # [128, (s b)]

    zq8v = np.asarray(Z_q.astype(np.float32) * F8SCALE, dtype=e4)  # [B, C]
    zq8 = np.ascontiguousarray(
        zq8v.T.reshape(2, 2, 128, B).transpose(2, 0, 1, 3)
    ).reshape(128, 4 * B)  # [128, (s i b)]

    qT = np.ascontiguousarray(queue.astype(np.float16).T)          # [512, 65536]
    cen8 = np.asarray(centroids.astype(np.float32) * F8SCALE, dtype=e4)

    in_maps = []
    for i in range(NCORES):
        q_sh = qT[:, i * QSH:(i + 1) * QSH]                        # [512, 8192]
        q_sh = np.ascontiguousarray(
            q_sh.reshape(KSUB, 128, QCHUNK, JW).transpose(1, 2, 0, 3)
        ).reshape(128, QCHUNK * KSUB * JW)  # [128, (h s jw)]
        q16 = np.concatenate([zqT, q_sh], axis=1)                  # [128, 33792]
        c_sh = np.zeros((2 * CPAIR * CW, C), e4)
        c_sh[:CSH] = cen8[i * CSH:(i + 1) * CSH]
        # [k, pair, j, s, i, col]: value C[(2p+j)*CW+col, s*256+i*128+k]
        c_sh = np.ascontiguousarray(
            c_sh.reshape(CPAIR, 2, CW, 2, 2, 128).transpose(5, 0, 1, 3, 4, 2)
        ).reshape(128, CPAIR * 8 * CW)  # [128, (p j s i col)]
        c8 = np.concatenate([zq8, c_sh], axis=1)                   # [128, 29696]
        in_maps.append({"c8": c8, "q16": q16})
    return in_maps


def kernel(Z_q, Z_k, queue, centroids, kmeans_temp, neg_raw):
    global last_exec_time_ns
    from concourse.bass_utils import run_bass_kernel_spmd

    nc = _get_nc()
    in_maps = _prep_inputs(Z_q, queue, centroids)

    trace = bool(int(os.environ.get("MOCO_BASS_TRACE", "0")))
    out = run_bass_kernel_spmd(nc, in_maps, core_ids=list(range(NCORES)), trace=trace)
    last_exec_time_ns = out.exec_time_ns
    res = out.results

    # ---- host combine (tiny) ----
    lp = (Z_q.astype(np.float64) * Z_k.astype(np.float64)).sum(axis=1)  # l_pos
    lp_t = lp / INFO_TEMP

    # part-1 loss: logsumexp over [l_pos | l_neg]/T per batch row
    S = np.zeros(B, np.float64)
    count = 0.0
    for r in res:
        S += r["p1"][0, :B].astype(np.float64)
        count += float(r["p1"][:, B].sum())
    S += np.exp(lp_t)
    lse1 = np.log(S)
    loss1 = np.mean(lse1 - lp_t)

    # accuracy: count of columns where batch row 0 attains the max
    count += float(lp[0] >= lp.max())
    accuracy = count / (1 + QUEUE)

    # part-2: global argmax over centroids (== argmin of ||c||^2 - 2 s)
    s2_full = np.empty((B, NCL), np.float32)
    for i, r in enumerate(res):
        sh = r["s2"].reshape(B, S2W).astype(np.float32) / S2SCALE
        s2_full[:, i * CSH:(i + 1) * CSH] = sh[:, :CSH]

    I = np.argmax(s2_full, axis=1)                              # first-index ties
    maxv = s2_full[np.arange(B), I].astype(np.float64)

    kt = kmeans_temp.astype(np.float64)
    pl_pos = maxv / kt[I]                                       # [B]
    neg_idx = neg_raw + (neg_raw >= I[:, None]).astype(neg_raw.dtype)
    pl_neg = (
        np.take_along_axis(s2_full, neg_idx, axis=1).astype(np.float64)
        / kt[neg_idx]
    )
    plogits = np.concatenate([pl_pos[:, None], pl_neg], axis=1)
    m = plogits.max(axis=1)
    plse = np.log(np.exp(plogits - m[:, None]).sum(axis=1)) + m
    ploss = np.mean(plse - pl_pos)

    loss = loss1 + PROTO_FACTOR * ploss
    return np.float32(loss), np.float32(accuracy)


# revision 70
# speedup vs baseline: 1.1782x; 1.1782x over previous
"""MoCo loss (InfoNCE over a 65536-entry queue + proto-NCE over 50000
k-means centroids) on 8 Trainium2 NeuronCores.

Strategy: the heavy work is two matmuls, Z_q @ queue.T (256x512x65536)
and Z_q @ centroids.T (256x512x50000).  We shard the tables by row
across the 8 cores and replicate Z_q.

Per core:
  part 2 (centroid shard, batch -> partitions, centroids -> free):
    - fp8 e4m3 (scale 64) inputs, DoubleRow matmuls (2x PE throughput)
    - s2*4096 exported to DRAM as fp16 (argmax + exclusion gather on
      host; centroids are L2-normalized so argmin ||c||^2 - 2 s ==
      argmax s).  e4m3 shifts the proto-term argmax for ~30/256 rows
      (near-ties); measured end-to-end loss impact ~1e-3 relative.
  part 1 (queue shard, fp16, queue rows -> partitions, batch -> free):
    - s1 = q_shard @ Z_q.T                      (PE)
    - row-max + (s1[:,0] >= rowmax) on fp32 PSUM -> exact accuracy
    - exp(s1/T) to fp16 (no shift needed: s/T in [-4, 4])      (ACT)
    - ones-matmul partition sum -> per-batch partial sum-of-exp (PE)

All DMA goes through the sync-engine HWDGE ring (the gpsimd SWDGE path
costs ~2.7us of Q7 descriptor generation per transfer and serializes).
A burst of dummy ones-matmuls at the head warms the PE HAM clock gate
(cold PE runs at 1.2 GHz for the first ~3.4us of activity) while the
first DMAs are still in flight.

The host combines the tiny per-core partials (logsumexp merge, accuracy
count, global argmax, 513-wide proto softmax).
"""

import os
import numpy as np
import ml_dtypes

B, C = 256, 512
QUEUE, NCL, NNEG = 65536, 50000, 512
INFO_TEMP = 0.07
PROTO_FACTOR = 0.5
NCORES = 8
QSH = QUEUE // NCORES          # 8192 queue rows per core
CSH = NCL // NCORES            # 6250 centroid rows per core
CCH = 13                       # cT matmul chunks
CW = 512                       # cols per chunk (last chunk computes 128)
CWL = 128                      # last-chunk matmul width (6250 <= 12*512+128)
S2W = 12 * CW + CWL            # 6272 exported s2 columns
CPAIR = 7                      # cT DMA transfers (2 chunks each, 14th is pad)
KSUB = C // 128                # 4 fp16 contraction subtiles
NT = 32                        # part-1 tiles (256 queue rows each)
QCHUNK = 8                     # qT DMA chunks (1 MiB each)
JW = QSH // QCHUNK             # 1024
F8SCALE = 64.0                 # e4m3 quantization scale
S2SCALE = F8SCALE * F8SCALE    # s2 output carries this factor
NWARM = 32                     # PE warmup matmuls: must keep the PE busy
                               # until part-2 data lands (~11us) — an idle
                               # gap resets the HAM activity window and the
                               # first ~3.4us of part 2 would run at 1.2GHz
# tiles whose subtile-1 sum also goes to gpsimd (spread out: two adds on
# one tile costs gpsimd 1.34us vs the PE's 0.87us/tile cadence)
GP_Q1 = frozenset(range(0, 30, 3))

_CACHE = {}

# exec time of the last device run (ns), populated when tracing is on
last_exec_time_ns = None


def _build():
    import concourse.bass as bass
    import concourse.tile as tile
    from concourse import bacc, mybir

    dt = mybir.dt
    DR = mybir.MatmulPerfMode.DoubleRow
    nc = bacc.Bacc(
        "TRN2", target_bir_lowering=False, debug=False, num_devices=NCORES
    )

    # ---- DRAM I/O (partition-major so every DMA is a flat [128, N]) ----
    # Each extra DMA stalls the HWDGE ring ~0.9us for its completion-sem
    # write, so zq8 rides with cT pair 0 and zqT with qT chunk 0.
    c8_d = nc.dram_tensor(
        "c8", [128, 4 * B + CPAIR * 8 * CW], dt.float8e4, kind="ExternalInput"
    ).ap()  # zq8 [s,i,b] ++ per-pair [j(chunk),s,i,col] blocks
    q16_d = nc.dram_tensor(
        "q16", [128, KSUB * B + QCHUNK * KSUB * JW], dt.float16,
        kind="ExternalInput"
    ).ap()  # zqT [s,b] ++ qT chunks [s,jw] blocks

    # [128, B+1]: cols 0..B-1 = per-batch partial sum-of-exp (all rows
    # identical); col B = per-partition accuracy count.  One flat DMA —
    # a separate [128]-column output pays a ~6us completion-latency tail.
    p1_d = nc.dram_tensor("p1", [128, B + 1], dt.float32, kind="ExternalOutput").ap()
    s2_d = nc.dram_tensor(
        "s2", [2, 128, S2W], dt.float16, kind="ExternalOutput"
    ).ap()

    with tile.TileContext(nc) as tc:
        with (
            tc.tile_pool(name="const", bufs=1) as cpool,
            tc.tile_pool(name="work", bufs=4) as wpool,
            tc.tile_pool(name="ps1", bufs=3, space="PSUM") as ps1,
            tc.tile_pool(name="psum1s", bufs=1, space="PSUM") as ps1s,
            tc.tile_pool(name="ps2", bufs=4, space="PSUM") as ps2,
        ):
            # ---- constants + PE warmup (runs while DMAs stream) ----
            ones_sb = cpool.tile([128, 128], dt.float16)
            nc.vector.memset(ones_sb[:], 1.0)
            # warmup PSUM lives in the (part-1) ps1 pool so it doesn't eat
            # one of part-2's four copy-recycle slots
            warm_ps = ps1.tile([128, 512], dt.float32, tag="s1")
            for _ in range(NWARM):
                nc.tensor.matmul(warm_ps[:, :128], ones_sb[:], ones_sb[:],
                                 start=True, stop=True)

            # ---- resident SBUF tensors; all loads on the sync HWDGE ring.
            # Ring order == HBM priority: part-2 operands first, then the
            # part-1 operands, then (programmatically later) the outputs.
            NC0 = 4 * B + 8 * CW           # zq8 + cT pair 0
            NS0 = 4 * B + 4 * CW           # zq8 + chunk 0 only
            c0_sb = cpool.tile([128, NC0], dt.float8e4)
            # the first transfer runs at ramp rate (~150-250 GB/s), so land
            # zq8+chunk0 first and chunk1 as its own transfer
            nc.sync.dma_start(c0_sb[:, :NS0], c8_d[:, :NS0])
            nc.sync.dma_start(c0_sb[:, NS0:NC0], c8_d[:, NS0:NC0])
            zq8_sb = c0_sb[:, :4 * B].rearrange(
                "p (s i b) -> p s i b", s=2, i=2)
            cT8_sb = [c0_sb[:, 4 * B:].rearrange(
                "p (j s i w) -> p j s i w", j=2, s=2, i=2)]
            for p in range(1, CPAIR):
                t = cpool.tile([128, 8 * CW], dt.float8e4, tag=f"cT{p}")
                off = 4 * B + p * 8 * CW
                nc.sync.dma_start(t[:], c8_d[:, off:off + 8 * CW])
                cT8_sb.append(t.rearrange(
                    "p (j s i w) -> p j s i w", j=2, s=2, i=2))

            NQ0 = KSUB * B + KSUB * JW     # zqT + qT chunk 0
            q0_sb = cpool.tile([128, NQ0], dt.float16)
            nc.sync.dma_start(q0_sb[:], q16_d[:, :NQ0])
            zqT_sb = q0_sb[:, :KSUB * B].rearrange("p (s b) -> p s b", s=KSUB)
            qt_sb = [q0_sb[:, KSUB * B:].rearrange("p (s j) -> p s j", s=KSUB)]
            for h in range(1, QCHUNK):
                t = cpool.tile([128, KSUB, JW], dt.float16, tag=f"qt{h}")
                off = KSUB * B + h * KSUB * JW
                nc.sync.dma_start(t[:], q16_d[:, off:off + KSUB * JW])
                qt_sb.append(t)

            # ---- part 2: centroid shard, fp8 DoubleRow (argmax on host) ----
            s2_sb = cpool.tile([128, 2, S2W], dt.float16)

            # N=512 puts the DoubleRow moving operand at its 1024-element
            # max; each matmul output fills exactly one PSUM bank.  The
            # last chunk only has 106 live columns, so it runs at N=128.
            for ch in range(CCH):
                w = CW if ch < CCH - 1 else CWL
                col = ch * CW
                for bt in range(2):
                    s2_ps = ps2.tile([128, CW], dt.float32, tag="s2")
                    for s in range(2):
                        nc.tensor.matmul(
                            s2_ps[:, :w],
                            zq8_sb[:, s, :, bt * 128:(bt + 1) * 128],
                            cT8_sb[ch // 2][:, ch % 2, s, :, :w],
                            start=(s == 0),
                            stop=(s == 1),
                            perf_mode=DR,
                        )
                    # cast to fp16; strict DVE/ACT alternation (bt toggles
                    # every group — (ch+bt)%2 would put two consecutive
                    # groups on the same engine and stall the PSUM recycle)
                    if bt == 0:
                        nc.vector.tensor_copy(
                            s2_sb[:, bt, col:col + w], s2_ps[:, :w]
                        )
                    else:
                        nc.scalar.activation(
                            s2_sb[:, bt, col:col + w],
                            s2_ps[:, :w],
                            mybir.ActivationFunctionType.Copy,
                        )
            for bt in range(2):
                nc.sync.dma_start(s2_d[bt], s2_sb[:, bt])

            # ---- part 1: queue shard, 32 tiles of 256 rows ----
            rm_buf = cpool.tile([128, NT, 2], dt.float32)  # per-subtile row max
            s0_buf = cpool.tile([128, NT, 2], dt.float32)  # s1[:, batch 0]
            p1s_ps = ps1s.tile([128, B], dt.float32)       # sum-of-exp accum
            # subtile-0 exp sums accumulate on the (otherwise idle) gpsimd
            # engine; only subtile 1 goes through PE ones-matmuls.  fp16
            # partials stay < 64 per entry, so precision is ample.
            gacc = cpool.tile([128, B], dt.float16)
            nc.gpsimd.memset(gacc[:], 0.0)

            # ones-matmuls run two tiles behind the s1 matmuls so the
            # in-order PE never waits for ACT's exp of a recent tile
            LAG = 2
            exp_tiles = [None] * NT
            for t in range(NT):
                s1_ps = ps1.tile([128, 2, B], dt.float32, tag="s1")
                for q in range(2):
                    jt = t * 2 + q
                    h, off = divmod(jt * 128, JW)
                    for s in range(KSUB):
                        nc.tensor.matmul(
                            s1_ps[:, q, :],
                            qt_sb[h][:, s, off:off + 128],
                            zqT_sb[:, s, :],
                            start=(s == 0),
                            stop=(s == KSUB - 1),
                        )
                exp_t = wpool.tile([128, 2, B], dt.float16, tag="exp")
                exp_tiles[t] = exp_t
                nc.scalar.activation(
                    exp_t[:],
                    s1_ps[:],
                    mybir.ActivationFunctionType.Exp,
                    scale=1.0 / INFO_TEMP,
                )
                if t < NT - LAG:
                    # last LAG tiles' subtile sums go through the PE
                    # instead, so the final ones-matmul never waits on
                    # the trailing gpsimd chain.  gpsimd also absorbs the
                    # subtile-1 sums of the first NGQ1 tiles (engine
                    # balance: PE ~30.6us, gpsimd ~26.8us over part 1).
                    nc.gpsimd.tensor_add(gacc[:], gacc[:], exp_t[:, 0, :])
                    if t in GP_Q1:
                        nc.gpsimd.tensor_add(gacc[:], gacc[:], exp_t[:, 1, :])
                u = t - LAG
                if u >= 0 and u not in GP_Q1:
                    nc.tensor.matmul(
                        p1s_ps[:],
                        ones_sb[:],
                        exp_tiles[u][:, 1, :],
                        start=(u == 1),
                        stop=False,
                    )
                nc.vector.tensor_reduce(
                    rm_buf[:, t, :],
                    s1_ps[:],
                    axis=mybir.AxisListType.X,
                    op=mybir.AluOpType.max,
                )
                nc.vector.tensor_copy(s0_buf[:, t, :], s1_ps[:, :, 0])
            # tail: last LAG tiles' sums (they wait on the final exps), then
            # the gacc pass last — its wait on the trailing gpsimd add chain
            # overlaps the exp waits above
            for tl in range(NT - LAG, NT):
                for q in range(2):
                    nc.tensor.matmul(
                        p1s_ps[:],
                        ones_sb[:],
                        exp_tiles[tl][:, q, :],
                        start=False,
                        stop=False,
                    )
            nc.tensor.matmul(
                p1s_ps[:], ones_sb[:], gacc[:], start=False, stop=True
            )

            ge_buf = cpool.tile([128, NT * 2], dt.float32)
            nc.vector.tensor_tensor(
                ge_buf[:],
                s0_buf[:].rearrange("p t q -> p (t q)"),
                rm_buf[:].rearrange("p t q -> p (t q)"),
                mybir.AluOpType.is_ge,
            )
            p1_sb = cpool.tile([128, B + 1], dt.float32)
            nc.vector.tensor_reduce(
                p1_sb[:, B:], ge_buf[:], axis=mybir.AxisListType.X,
                op=mybir.AluOpType.add
            )
            nc.vector.tensor_copy(p1_sb[:, :B], p1s_ps[:])
            nc.sync.dma_start(p1_d[:], p1_sb[:])

    nc.compile()
    return nc


def _get_nc():
    if "nc" not in _CACHE:
        _CACHE["nc"] = _build()
    return _CACHE["nc"]


def _prep_inputs(Z_q, queue, centroids):
    """Host-side shard prep.

    fp16 tensors go partition-major with the contraction dim C split in
    KSUB=4 blocks of 128 (c = s*128 + k).  fp8 tensors use DoubleRow
    pairing [k, s, i, col] with c = s*256 + i*128 + k on BOTH operands.
    """
    e4 = ml_dtypes.float8_e4m3
    zqT = np.ascontiguousarray(
        Z_q.astype(np.float16).T.reshape(KSUB, 128, B).transpose(1, 0, 2)
    ).reshape(128, KSUB * B)  # [128, (s b)]

    zq8v = np.asarray(Z_q.astype(np.float32) * F8SCALE, dtype=e4)  # [B, C]
    zq8 = np.ascontiguousarray(
        zq8v.T.reshape(2, 2, 128, B).transpose(2, 0, 1, 3)
    ).reshape(128, 4 * B)  # [128, (s i b)]

    qT = np.ascontiguousarray(queue.astype(np.float16).T)          # [512, 65536]
    cen8 = np.asarray(centroids.astype(np.float32) * F8SCALE, dtype=e4)

    in_maps = []
    for i in range(NCORES):
        q_sh = qT[:, i * QSH:(i + 1) * QSH]                        # [512, 8192]
        q_sh = np.ascontiguousarray(
            q_sh.reshape(KSUB, 128, QCHUNK, JW).transpose(1, 2, 0, 3)
        ).reshape(128, QCHUNK * KSUB * JW)  # [128, (h s jw)]
        q16 = np.concatenate([zqT, q_sh], axis=1)                  # [128, 33792]
        c_sh = np.zeros((2 * CPAIR * CW, C), e4)
        c_sh[:CSH] = cen8[i * CSH:(i + 1) * CSH]
        # [k, pair, j, s, i, col]: value C[(2p+j)*CW+col, s*256+i*128+k]
        c_sh = np.ascontiguousarray(
            c_sh.reshape(CPAIR, 2, CW, 2, 2, 128).transpose(5, 0, 1, 3, 4, 2)
        ).reshape(128, CPAIR * 8 * CW)  # [128, (p j s i col)]
        c8 = np.concatenate([zq8, c_sh], axis=1)                   # [128, 29696]
        in_maps.append({"c8": c8, "q16": q16})
    return in_maps


def kernel(Z_q, Z_k, queue, centroids, kmeans_temp, neg_raw):
    global last_exec_time_ns
    from concourse.bass_utils import run_bass_kernel_spmd

    nc = _get_nc()
    in_maps = _prep_inputs(Z_q, queue, centroids)

    trace = bool(int(os.environ.get("MOCO_BASS_TRACE", "0")))
    out = run_bass_kernel_spmd(nc, in_maps, core_ids=list(range(NCORES)), trace=trace)
    last_exec_time_ns = out.exec_time_ns
    res = out.results

    # ---- host combine (tiny) ----
    lp = (Z_q.astype(np.float64) * Z_k.astype(np.float64)).sum(axis=1)  # l_pos
    lp_t = lp / INFO_TEMP

    # part-1 loss: logsumexp over [l_pos | l_neg]/T per batch row
    S = np.zeros(B, np.float64)
    count = 0.0
    for r in res:
        S += r["p1"][0, :B].astype(np.float64)
        count += float(r["p1"][:, B].sum())
    S += np.exp(lp_t)
    lse1 = np.log(S)
    loss1 = np.mean(lse1 - lp_t)

    # accuracy: count of columns where batch row 0 attains the max
    count += float(lp[0] >= lp.max())
    accuracy = count / (1 + QUEUE)

    # part-2: global argmax over centroids (== argmin of ||c||^2 - 2 s)
    s2_full = np.empty((B, NCL), np.float32)
    for i, r in enumerate(res):
        sh = r["s2"].reshape(B, S2W).astype(np.float32) / S2SCALE
        s2_full[:, i * CSH:(i + 1) * CSH] = sh[:, :CSH]

    I = np.argmax(s2_full, axis=1)                              # first-index ties
    maxv = s2_full[np.arange(B), I].astype(np.float64)

    kt = kmeans_temp.astype(np.float64)
    pl_pos = maxv / kt[I]                                       # [B]
    neg_idx = neg_raw + (neg_raw >= I[:, None]).astype(neg_raw.dtype)
    pl_neg = (
        np.take_along_axis(s2_full, neg_idx, axis=1).astype(np.float64)
        / kt[neg_idx]
    )
    plogits = np.concatenate([pl_pos[:, None], pl_neg], axis=1)
    m = plogits.max(axis=1)
    plse = np.log(np.exp(plogits - m[:, None]).sum(axis=1)) + m
    ploss = np.mean(plse - pl_pos)

    loss = loss1 + PROTO_FACTOR * ploss
    return np.float32(loss), np.float32(accuracy)
